# revision 5
# baseline (speedup 1.0000x reference)
"""Trainium2 Bass kernel for DepthAggregateModule.

Reference computation (per position p = (b,t), D=2048, L=12, C=4, H=48):
  x_cur   = xs[L-1]
  dw      = gelu(rms(x_cur) @ w1) @ w2            # [pos, C*L]
  dw      = dw + static_weight  (flat col c*L+l)
  agg[c]  = sum_l dw[:, c*L+l] * (rms(xs_l) * pre_scales[l])   # [pos, D]
  out[c]  = rms(agg[c]) * post_scales[c] + x_cur

Strategy: shard the 4096 (b,t) positions across 8 cores (512 each).
Per core, 4 position-tiles of 128 (partition dim = position).  The
per-position l-contraction runs on the TensorEngine as diagonal
matmuls: lhsT = diag(dw[:, c*L+l] * rstd_l) [128x128] bf16, rhs =
xs_l chunk [128x512] bf16, accumulated over l in fp32 PSUM.  The rms
scale factors fold into the diagonals, so bulk xs data is touched only
by: casting DMA-in (f32->bf16 via SWDGE), one ACT square+accumulate
pass (row sum-of-squares), and the PE einsum stream.  Layer L-1 is
also loaded in f32 for the residual add and the dw matmul chain.
"""

import os
from contextlib import ExitStack

import numpy as np

import concourse.bacc as bacc
import concourse.bass as bass
import concourse.tile as tile
from concourse import mybir
from concourse.bass_utils import run_bass_kernel_spmd

F32 = mybir.dt.float32
BF16 = mybir.dt.bfloat16
AF = mybir.ActivationFunctionType
OP = mybir.AluOpType
AX = mybir.AxisListType

L, B, T, D = 12, 2, 2048, 2048
C, H = 4, 48
CL = C * L
EPS = 1e-6
NCORES = 8
P = 128                      # partition dim = positions per tile
NPOS = (B * T) // NCORES     # 512 positions per core
NTILES = NPOS // P           # 4
QS = 512                     # free-dim chunk (one fp32 PSUM bank)
NQ = D // QS                 # 4

_LAST_RESULTS = None         # BassKernelResults of the last run (for test.py)


def build_kernel(pre_ones: bool, post_ones: bool) -> bass.Bass:
    nc = bacc.Bacc("TRN2", target_bir_lowering=False, debug=False)

    xs_s = nc.dram_tensor("xs_s", [L, NPOS, D], F32, kind="ExternalInput")
    w1_t = nc.dram_tensor("w1", [D, H], F32, kind="ExternalInput")
    w2_t = nc.dram_tensor("w2", [H, CL], F32, kind="ExternalInput")
    stat_t = nc.dram_tensor("stat", [CL], F32, kind="ExternalInput")
    ident_t = nc.dram_tensor("ident", [P, P], F32, kind="ExternalInput")
    identb_t = nc.dram_tensor("identb", [P, P], BF16, kind="ExternalInput")
    pre_t = None if pre_ones else nc.dram_tensor("pre", [L, D], F32, kind="ExternalInput")
    post_t = None if post_ones else nc.dram_tensor("post", [C, D], F32, kind="ExternalInput")
    out_s = nc.dram_tensor("out_s", [C, NPOS, D], F32, kind="ExternalOutput")

    with ExitStack() as ctx:
        tc = ctx.enter_context(tile.TileContext(nc))

        consts = ctx.enter_context(tc.tile_pool(name="consts", bufs=1))
        xs_pool = ctx.enter_context(tc.tile_pool(name="xs", bufs=2))
        sm_pool = ctx.enter_context(tc.tile_pool(name="small", bufs=2))
        diag_pool = ctx.enter_context(tc.tile_pool(name="diag", bufs=1))
        aggsb_pool = ctx.enter_context(tc.tile_pool(name="aggsb", bufs=1))
        outsb_pool = ctx.enter_context(tc.tile_pool(name="outsb", bufs=6))
        scr_pool = ctx.enter_context(tc.tile_pool(name="scr", bufs=1))
        agg_ps = ctx.enter_context(tc.tile_pool(name="aggps", bufs=2, space="PSUM"))
        pt_ps = ctx.enter_context(tc.tile_pool(name="ptps", bufs=2, space="PSUM"))
        dw_ps = ctx.enter_context(tc.tile_pool(name="dwps", bufs=1, space="PSUM"))

        # ---- constants -------------------------------------------------
        ident_sb = consts.tile([P, P], F32, name="ident_sb")
        nc.sync.dma_start(out=ident_sb, in_=ident_t[:, :])
        identb_sb = consts.tile([P, P], BF16, name="identb_sb")
        nc.sync.dma_start(out=identb_sb, in_=identb_t[:, :])

        w1s = consts.tile([P, D // P, H], F32, name="w1s")
        nc.sync.dma_start(
            out=w1s, in_=w1_t[:, :].rearrange("(k p) h -> p k h", p=P)
        )

        w2s = consts.tile([H, CL], F32, name="w2s")
        nc.sync.dma_start(out=w2s, in_=w2_t[:, :])

        stat_bc = consts.tile([P, CL], F32, name="stat_bc")
        stat_ap = stat_t[:]
        nc.sync.dma_start(
            out=stat_bc,
            in_=bass.AP(tensor=stat_ap.tensor, offset=stat_ap.offset,
                        ap=[[0, P]] + stat_ap.ap),
        )

        eps_sb = consts.tile([P, 1], F32, name="eps_sb")
        nc.vector.memset(eps_sb, EPS)

        pre_bc = None
        if pre_t is not None:
            # [128, L, D] replicated across partitions: 12 MB — fallback only.
            pre_bc = consts.tile([P, L, D], F32, name="pre_bc")
            pre_ap = pre_t[:, :]
            nc.sync.dma_start(
                out=pre_bc,
                in_=bass.AP(tensor=pre_ap.tensor, offset=pre_ap.offset,
                            ap=[[0, P]] + pre_ap.ap),
            )
        post_bc = None
        if post_t is not None:
            post_bc = consts.tile([P, C, D], F32, name="post_bc")
            post_ap = post_t[:, :]
            nc.sync.dma_start(
                out=post_bc,
                in_=bass.AP(tensor=post_ap.tensor, offset=post_ap.offset,
                            ap=[[0, P]] + post_ap.ap),
            )

        scr = scr_pool.tile([P, QS], F32, name="scr")  # ACT dump target

        # ---- per position-tile ----------------------------------------
        for it in range(NTILES):
            p0 = it * P
            sfx = f"_{it}"

            xb = [None] * L          # bf16 [P, D] per layer (einsum rhs)
            rstd = sm_pool.tile([P, L], F32, tag="rstd", name="rstd" + sfx)
            rsr = sm_pool.tile([P, L], F32, tag="rsr", name="rsr" + sfx)

            def rstd_from_sq(l, sq):
                # rstd_l = 1/sqrt(mean(x^2) + eps)
                nc.vector.tensor_reduce(out=rsr[:, l:l + 1], in_=sq,
                                        axis=AX.X, op=OP.add)
                nc.scalar.activation(out=rsr[:, l:l + 1], in_=rsr[:, l:l + 1],
                                     func=AF.Sqrt, bias=eps_sb, scale=1.0 / D)
                nc.vector.reciprocal(out=rstd[:, l:l + 1], in_=rsr[:, l:l + 1])

            # --- layer L-1 in f32 (residual + dw chain), plus bf16 copy.
            x11q = []
            sq11 = sm_pool.tile([P, NQ], F32, tag="sq11", name="sq11" + sfx)
            xb11 = xs_pool.tile([P, D], BF16, tag="xb11", name="xb11" + sfx)
            for q in range(NQ):
                t = xs_pool.tile([P, QS], F32, tag=f"x11_{q}",
                                 name=f"x11_{q}" + sfx)
                nc.sync.dma_start(
                    out=t, in_=xs_s[L - 1, p0:p0 + P, q * QS:(q + 1) * QS])
                nc.scalar.activation(out=scr, in_=t, func=AF.Square,
                                     accum_out=sq11[:, q:q + 1])
                nc.vector.tensor_copy(out=xb11[:, q * QS:(q + 1) * QS], in_=t)
                x11q.append(t)
            xb[L - 1] = xb11
            rstd_from_sq(L - 1, sq11)

            # Transpose x_cur into [d, pos] chunks for the w1 matmul.
            x11T = sm_pool.tile([P, D // P, P], F32, tag="x11T",
                                name="x11T" + sfx)
            for q in range(NQ):
                pt = pt_ps.tile([P, QS], F32, tag="pt", name=f"pt{q}" + sfx)
                for j in range(4):
                    k = q * 4 + j
                    nc.tensor.matmul(
                        pt[:, j * P:(j + 1) * P],
                        x11q[q][:, j * P:(j + 1) * P],
                        ident_sb, is_transpose=True, skip_group_check=True)
                    nc.vector.tensor_copy(out=x11T[:, k, :],
                                          in_=pt[:, j * P:(j + 1) * P])

            # dw chain: dw = gelu(rms(x_cur) @ w1) @ w2 + static
            pdw1 = dw_ps.tile([P, H], F32, tag="pdw1", name="pdw1" + sfx)
            for k in range(D // P):
                nc.tensor.matmul(pdw1, x11T[:, k, :], w1s[:, k, :],
                                 start=(k == 0), stop=(k == D // P - 1))
            h1 = sm_pool.tile([P, H], F32, tag="h1", name="h1" + sfx)
            nc.scalar.activation(out=h1, in_=pdw1, func=AF.Gelu,
                                 scale=rstd[:, L - 1:L])
            pth = pt_ps.tile([P, QS], F32, tag="pt", name="pth" + sfx)
            nc.tensor.matmul(pth[:H, :P], h1, ident_sb, is_transpose=True,
                             skip_group_check=True)
            h1T = sm_pool.tile([H, P], F32, tag="h1T", name="h1T" + sfx)
            nc.vector.tensor_copy(out=h1T, in_=pth[:H, :P])
            pdw2 = dw_ps.tile([P, CL], F32, tag="pdw2", name="pdw2" + sfx)
            nc.tensor.matmul(pdw2, h1T, w2s, start=True, stop=True)
            dwf = sm_pool.tile([P, CL], F32, tag="dwf", name="dwf" + sfx)
            nc.vector.scalar_tensor_tensor(out=dwf, in0=pdw2, scalar=1.0,
                                           in1=stat_bc, op0=OP.mult,
                                           op1=OP.add)

            # --- layers 0..L-2: casting DMA straight to bf16.
            for l in range(L - 1):
                xbl = xs_pool.tile([P, D], BF16, tag=f"xb{l}",
                                   name=f"xb{l}" + sfx)
                nc.gpsimd.dma_start(out=xbl, in_=xs_s[l, p0:p0 + P, :])
                sq = sm_pool.tile([P, NQ], F32, tag=f"sq{l}",
                                  name=f"sq{l}" + sfx, bufs=1)
                for q in range(NQ):
                    nc.scalar.activation(out=scr,
                                         in_=xbl[:, q * QS:(q + 1) * QS],
                                         func=AF.Square,
                                         accum_out=sq[:, q:q + 1])
                rstd_from_sq(l, sq)
                xb[l] = xbl

            # Fallback: apply pre_scales in place (after the sq-accum).
            if pre_bc is not None:
                for l in range(L):
                    nc.vector.tensor_mul(xb[l], xb[l], pre_bc[:, l, :])

            # Diagonals: diag_cl = ident * dw[:, c*L+l] * rstd_l  (bf16)
            diag = [[None] * L for _ in range(C)]
            for c in range(C):
                for l in range(L):
                    dg = diag_pool.tile([P, P], BF16, tag=f"dg{c}_{l}",
                                        name=f"dg{c}_{l}" + sfx)
                    nc.vector.tensor_scalar(
                        out=dg, in0=identb_sb,
                        scalar1=dwf[:, c * L + l:c * L + l + 1],
                        scalar2=rstd[:, l:l + 1],
                        op0=OP.mult, op1=OP.mult)
                    diag[c][l] = dg

            # Einsum: agg[c] = sum_l diag_cl @ xs_l, then post-RMS + residual.
            aggsq = [sm_pool.tile([P, NQ], F32, tag=f"asq{c}",
                                  name=f"asq{c}" + sfx) for c in range(C)]
            aggsb = {}
            for cp in range(2):
                for q in range(NQ):
                    for ci in range(2):
                        c = cp * 2 + ci
                        pagg = agg_ps.tile([P, QS], F32, tag=f"pagg{ci}",
                                           name=f"pagg{c}_{q}" + sfx)
                        for l in range(L):
                            nc.tensor.matmul(
                                pagg, diag[c][l],
                                xb[l][:, q * QS:(q + 1) * QS],
                                start=(l == 0), stop=(l == L - 1))
                        asb = aggsb_pool.tile([P, QS], F32, tag=f"as{ci}_{q}",
                                              name=f"as{c}_{q}" + sfx)
                        nc.vector.tensor_copy(out=asb, in_=pagg)
                        nc.scalar.activation(out=scr, in_=pagg, func=AF.Square,
                                             accum_out=aggsq[c][:, q:q + 1])
                        aggsb[(c, q)] = asb

            rstda = sm_pool.tile([P, C], F32, tag="rstda", name="rstda" + sfx)
            rsra = sm_pool.tile([P, C], F32, tag="rsra", name="rsra" + sfx)
            for c in range(C):
                nc.vector.tensor_reduce(out=rsra[:, c:c + 1], in_=aggsq[c],
                                        axis=AX.X, op=OP.add)
                nc.scalar.activation(out=rsra[:, c:c + 1], in_=rsra[:, c:c + 1],
                                     func=AF.Sqrt, bias=eps_sb, scale=1.0 / D)
                nc.vector.reciprocal(out=rstda[:, c:c + 1], in_=rsra[:, c:c + 1])

            for c in range(C):
                for q in range(NQ):
                    osb = outsb_pool.tile([P, QS], F32, tag="outsb",
                                          name=f"o{c}_{q}" + sfx)
                    if post_bc is None:
                        # out = agg * rstd_agg + x_cur
                        nc.vector.scalar_tensor_tensor(
                            out=osb, in0=aggsb[(c, q)],
                            scalar=rstda[:, c:c + 1], in1=x11q[q],
                            op0=OP.mult, op1=OP.add)
                    else:
                        nc.vector.tensor_scalar(
                            out=osb, in0=aggsb[(c, q)],
                            scalar1=rstda[:, c:c + 1], scalar2=None,
                            op0=OP.mult)
                        nc.vector.tensor_mul(
                            osb, osb, post_bc[:, c, q * QS:(q + 1) * QS])
                        nc.vector.tensor_add(osb, osb, x11q[q])
                    nc.sync.dma_start(
                        out=out_s[c, p0:p0 + P, q * QS:(q + 1) * QS], in_=osb)

    nc.compile()
    return nc


def _in_maps(inputs: dict) -> tuple[bass.Bass, list[dict]]:
    xs = np.ascontiguousarray(inputs["xs"], dtype=np.float32)
    w1 = np.ascontiguousarray(inputs["w1"], dtype=np.float32)
    w2 = np.ascontiguousarray(inputs["w2"], dtype=np.float32)
    stat = np.ascontiguousarray(inputs["static_weight"], dtype=np.float32)
    pre = np.ascontiguousarray(inputs["pre_scales"], dtype=np.float32)
    post = np.ascontiguousarray(inputs["post_scales"], dtype=np.float32)

    pre_ones = bool(np.all(pre == 1.0))
    post_ones = bool(np.all(post == 1.0))
    nc = build_kernel(pre_ones, post_ones)

    xs_flat = xs.reshape(L, B * T, D)
    import ml_dtypes
    ident = np.eye(P, dtype=np.float32)
    identb = np.eye(P, dtype=ml_dtypes.bfloat16)
    maps = []
    for k in range(NCORES):
        m = {
            "xs_s": np.ascontiguousarray(xs_flat[:, k * NPOS:(k + 1) * NPOS, :]),
            "w1": w1,
            "w2": w2,
            "stat": stat.reshape(CL),
            "ident": ident,
            "identb": identb,
        }
        if not pre_ones:
            m["pre"] = pre
        if not post_ones:
            m["post"] = post
        maps.append(m)
    return nc, maps


def kernel(**inputs) -> np.ndarray:
    global _LAST_RESULTS
    nc, maps = _in_maps(inputs)
    trace = os.environ.get("KERNEL_TRACE", "0") == "1"
    res = run_bass_kernel_spmd(nc, maps, core_ids=list(range(NCORES)),
                               trace=trace)
    _LAST_RESULTS = res
    out = np.concatenate([r["out_s"] for r in res.results], axis=1)
    return np.ascontiguousarray(out.reshape(C, B, T, D))


# revision 12
# speedup vs baseline: 1.1814x; 1.1814x over previous
"""Trainium2 Bass kernel for DepthAggregateModule.

Reference computation (per position p = (b,t), D=2048, L=12, C=4, H=48):
  x_cur   = xs[L-1]
  dw      = gelu(rms(x_cur) @ w1) @ w2            # [pos, C*L]
  dw      = dw + static_weight  (flat col c*L+l)
  agg[c]  = sum_l dw[:, c*L+l] * (rms(xs_l) * pre_scales[l])   # [pos, D]
  out[c]  = rms(agg[c]) * post_scales[c] + x_cur

Strategy: shard the 4096 (b,t) positions across 8 cores (512 each).
Per core, 4 position-tiles of 128 (partition dim = position).  The
per-position l-contraction runs on the TensorEngine as diagonal
matmuls: lhsT = diag(dw[:, c*L+l] * rstd_l) [128x128] bf16, rhs =
xs_l chunk [128x512] bf16, accumulated over l in fp32 PSUM.  The rms
scale factors fold into the diagonals, so bulk xs data is touched only
by: casting DMA-in (f32->bf16 via SWDGE), one ACT square+accumulate
pass (row sum-of-squares), and the PE einsum stream.  Layer L-1 is
also loaded in f32 for the residual add and the dw matmul chain.
"""

import os
from contextlib import ExitStack

import numpy as np

import concourse.bacc as bacc
import concourse.bass as bass
import concourse.tile as tile
from concourse import mybir
from concourse.bass_utils import run_bass_kernel_spmd

F32 = mybir.dt.float32
BF16 = mybir.dt.bfloat16
AF = mybir.ActivationFunctionType
OP = mybir.AluOpType
AX = mybir.AxisListType

L, B, T, D = 12, 2, 2048, 2048
C, H = 4, 48
CL = C * L
EPS = 1e-6
NCORES = 8
P = 128                      # partition dim = positions per tile
NPOS = (B * T) // NCORES     # 512 positions per core
NTILES = NPOS // P           # 4
QS = 512                     # free-dim chunk (one fp32 PSUM bank)
NQ = D // QS                 # 4

_LAST_RESULTS = None         # BassKernelResults of the last run (for test.py)


def build_kernel(pre_ones: bool, post_ones: bool) -> bass.Bass:
    nc = bacc.Bacc("TRN2", target_bir_lowering=False, debug=False)

    xs_s = nc.dram_tensor("xs_s", [L, NPOS, D], F32, kind="ExternalInput")
    w1_t = nc.dram_tensor("w1", [D, H], F32, kind="ExternalInput")
    w2_t = nc.dram_tensor("w2", [H, CL], F32, kind="ExternalInput")
    stat_t = nc.dram_tensor("stat", [CL], F32, kind="ExternalInput")
    ident_t = nc.dram_tensor("ident", [P, P], F32, kind="ExternalInput")
    identb_t = nc.dram_tensor("identb", [P, P], BF16, kind="ExternalInput")
    pre_t = None if pre_ones else nc.dram_tensor("pre", [L, D], F32, kind="ExternalInput")
    post_t = None if post_ones else nc.dram_tensor("post", [C, D], F32, kind="ExternalInput")
    out_s = nc.dram_tensor("out_s", [C, NPOS, D], F32, kind="ExternalOutput")

    with ExitStack() as ctx:
        tc = ctx.enter_context(tile.TileContext(nc))

        consts = ctx.enter_context(tc.tile_pool(name="consts", bufs=1))
        xs_pool = ctx.enter_context(tc.tile_pool(name="xs", bufs=2))
        sm_pool = ctx.enter_context(tc.tile_pool(name="small", bufs=2))
        diag_pool = ctx.enter_context(tc.tile_pool(name="diag", bufs=2))
        aggsb_pool = ctx.enter_context(tc.tile_pool(name="aggsb", bufs=1))
        outsb_pool = ctx.enter_context(tc.tile_pool(name="outsb", bufs=3))
        scr_pool = ctx.enter_context(tc.tile_pool(name="scr", bufs=1))
        agg_ps = ctx.enter_context(tc.tile_pool(name="aggps", bufs=2, space="PSUM"))
        pt_ps = ctx.enter_context(tc.tile_pool(name="ptps", bufs=2, space="PSUM"))
        dw_ps = ctx.enter_context(tc.tile_pool(name="dwps", bufs=1, space="PSUM"))

        # ---- constants -------------------------------------------------
        ident_sb = consts.tile([P, P], F32, name="ident_sb")
        nc.sync.dma_start(out=ident_sb, in_=ident_t[:, :])
        identb_sb = consts.tile([P, P], BF16, name="identb_sb")
        nc.sync.dma_start(out=identb_sb, in_=identb_t[:, :])

        w1s = consts.tile([P, D // P, H], F32, name="w1s")
        nc.sync.dma_start(
            out=w1s, in_=w1_t[:, :].rearrange("(k p) h -> p k h", p=P)
        )

        w2s = consts.tile([H, CL], F32, name="w2s")
        nc.sync.dma_start(out=w2s, in_=w2_t[:, :])

        stat_bc = consts.tile([P, CL], F32, name="stat_bc")
        stat_ap = stat_t[:]
        nc.sync.dma_start(
            out=stat_bc,
            in_=bass.AP(tensor=stat_ap.tensor, offset=stat_ap.offset,
                        ap=[[0, P]] + stat_ap.ap),
        )

        eps_sb = consts.tile([P, 1], F32, name="eps_sb")
        nc.vector.memset(eps_sb, EPS)

        pre_bc = None
        if pre_t is not None:
            # [128, L, D] replicated across partitions: 12 MB — fallback only.
            pre_bc = consts.tile([P, L, D], F32, name="pre_bc")
            pre_ap = pre_t[:, :]
            nc.sync.dma_start(
                out=pre_bc,
                in_=bass.AP(tensor=pre_ap.tensor, offset=pre_ap.offset,
                            ap=[[0, P]] + pre_ap.ap),
            )
        post_bc = None
        if post_t is not None:
            post_bc = consts.tile([P, C, D], F32, name="post_bc")
            post_ap = post_t[:, :]
            nc.sync.dma_start(
                out=post_bc,
                in_=bass.AP(tensor=post_ap.tensor, offset=post_ap.offset,
                            ap=[[0, P]] + post_ap.ap),
            )

        HS = 1024                                       # sq-accum chunk
        scr = scr_pool.tile([P, HS], F32, name="scr")   # ACT dump target

        # ---- per position-tile ----------------------------------------
        for it in range(NTILES):
            p0 = it * P
            sfx = f"_{it}"

            xb = [None] * L          # bf16 [P, D] per layer (einsum rhs)
            rstd = sm_pool.tile([P, L], F32, tag="rstd", name="rstd" + sfx)
            rsr = sm_pool.tile([P, L], F32, tag="rsr", name="rsr" + sfx)

            def rstd_from_sq(l, sq):
                # rstd_l = 1/sqrt(mean(x^2) + eps)
                nc.vector.tensor_reduce(out=rsr[:, l:l + 1], in_=sq,
                                        axis=AX.X, op=OP.add)
                nc.scalar.activation(out=rsr[:, l:l + 1], in_=rsr[:, l:l + 1],
                                     func=AF.Sqrt, bias=eps_sb, scale=1.0 / D)
                nc.vector.reciprocal(out=rstd[:, l:l + 1], in_=rsr[:, l:l + 1])

            # --- layer L-1 in f32 (residual + dw chain), plus bf16 copy.
            sq11 = sm_pool.tile([P, 2], F32, tag="sq11", name="sq11" + sfx)
            xb11 = xs_pool.tile([P, D], BF16, tag="xb11", name="xb11" + sfx)
            x11 = xs_pool.tile([P, D], F32, tag="x11", name="x11" + sfx)
            nc.sync.dma_start(out=x11, in_=xs_s[L - 1, p0:p0 + P, :])
            for h in range(2):
                nc.scalar.activation(out=scr, in_=x11[:, h * HS:(h + 1) * HS],
                                     func=AF.Square,
                                     accum_out=sq11[:, h:h + 1])
            nc.vector.tensor_copy(out=xb11, in_=x11)
            xb[L - 1] = xb11
            rstd_from_sq(L - 1, sq11)

            # Transpose x_cur into [d, pos] chunks for the w1 matmul.
            x11T = sm_pool.tile([P, D // P, P], F32, tag="x11T",
                                name="x11T" + sfx)
            for q in range(NQ):
                pt = pt_ps.tile([P, QS], F32, tag="pt", name=f"pt{q}" + sfx)
                for j in range(4):
                    k = q * 4 + j
                    nc.tensor.matmul(
                        pt[:, j * P:(j + 1) * P],
                        x11[:, k * P:(k + 1) * P],
                        ident_sb, is_transpose=True, skip_group_check=True)
                    nc.vector.tensor_copy(out=x11T[:, k, :],
                                          in_=pt[:, j * P:(j + 1) * P])

            # dw chain: dw = gelu(rms(x_cur) @ w1) @ w2 + static
            pdw1 = dw_ps.tile([P, H], F32, tag="pdw1", name="pdw1" + sfx)
            for k in range(D // P):
                nc.tensor.matmul(pdw1, x11T[:, k, :], w1s[:, k, :],
                                 start=(k == 0), stop=(k == D // P - 1))
            h1 = sm_pool.tile([P, H], F32, tag="h1", name="h1" + sfx)
            nc.scalar.activation(out=h1, in_=pdw1, func=AF.Gelu,
                                 scale=rstd[:, L - 1:L])
            pth = pt_ps.tile([P, QS], F32, tag="pt", name="pth" + sfx)
            nc.tensor.matmul(pth[:H, :P], h1, ident_sb, is_transpose=True,
                             skip_group_check=True)
            h1T = sm_pool.tile([H, P], F32, tag="h1T", name="h1T" + sfx)
            nc.vector.tensor_copy(out=h1T, in_=pth[:H, :P])
            pdw2 = dw_ps.tile([P, CL], F32, tag="pdw2", name="pdw2" + sfx)
            nc.tensor.matmul(pdw2, h1T, w2s, start=True, stop=True)
            dwf = sm_pool.tile([P, CL], F32, tag="dwf", name="dwf" + sfx)
            nc.vector.scalar_tensor_tensor(out=dwf, in0=pdw2, scalar=1.0,
                                           in1=stat_bc, op0=OP.mult,
                                           op1=OP.add)

            # --- layers 0..L-2: casting DMA straight to bf16.
            for l in range(L - 1):
                xbl = xs_pool.tile([P, D], BF16, tag=f"xb{l}",
                                   name=f"xb{l}" + sfx)
                nc.gpsimd.dma_start(out=xbl, in_=xs_s[l, p0:p0 + P, :])
                sq = sm_pool.tile([P, 2], F32, tag=f"sq{l}",
                                  name=f"sq{l}" + sfx, bufs=1)
                for h in range(2):
                    nc.scalar.activation(out=scr,
                                         in_=xbl[:, h * HS:(h + 1) * HS],
                                         func=AF.Square,
                                         accum_out=sq[:, h:h + 1])
                rstd_from_sq(l, sq)
                xb[l] = xbl

            # Fallback: apply pre_scales in place (after the sq-accum).
            if pre_bc is not None:
                for l in range(L):
                    nc.vector.tensor_mul(xb[l], xb[l], pre_bc[:, l, :])

            # Diagonals: diag_cl = ident * dw[:, c*L+l] * rstd_l  (bf16)
            diag = [[None] * L for _ in range(C)]
            for c in range(C):
                for l in range(L):
                    dg = diag_pool.tile([P, P], BF16, tag=f"dg{c}_{l}",
                                        name=f"dg{c}_{l}" + sfx)
                    nc.vector.tensor_scalar(
                        out=dg, in0=identb_sb,
                        scalar1=dwf[:, c * L + l:c * L + l + 1],
                        scalar2=rstd[:, l:l + 1],
                        op0=OP.mult, op1=OP.mult)
                    diag[c][l] = dg

            # Einsum: agg[c] = sum_l diag_cl @ xs_l, then post-RMS + residual.
            aggsq = [sm_pool.tile([P, NQ], F32, tag=f"asq{c}",
                                  name=f"asq{c}" + sfx) for c in range(C)]
            aggsb = {}
            for cp in range(2):
                for q in range(NQ):
                    for ci in range(2):
                        c = cp * 2 + ci
                        pagg = agg_ps.tile([P, QS], F32, tag=f"pagg{ci}",
                                           name=f"pagg{c}_{q}" + sfx)
                        for l in range(L):
                            nc.tensor.matmul(
                                pagg, diag[c][l],
                                xb[l][:, q * QS:(q + 1) * QS],
                                start=(l == 0), stop=(l == L - 1))
                        asb = aggsb_pool.tile([P, QS], F32, tag=f"as{ci}_{q}",
                                              name=f"as{c}_{q}" + sfx)
                        nc.vector.tensor_copy(out=asb, in_=pagg)
                        nc.scalar.activation(out=scr[:, :QS], in_=pagg,
                                             func=AF.Square,
                                             accum_out=aggsq[c][:, q:q + 1])
                        aggsb[(c, q)] = asb

            rstda = sm_pool.tile([P, C], F32, tag="rstda", name="rstda" + sfx)
            rsra = sm_pool.tile([P, C], F32, tag="rsra", name="rsra" + sfx)
            for c in range(C):
                nc.vector.tensor_reduce(out=rsra[:, c:c + 1], in_=aggsq[c],
                                        axis=AX.X, op=OP.add)
                nc.scalar.activation(out=rsra[:, c:c + 1], in_=rsra[:, c:c + 1],
                                     func=AF.Sqrt, bias=eps_sb, scale=1.0 / D)
                nc.vector.reciprocal(out=rstda[:, c:c + 1], in_=rsra[:, c:c + 1])

            for c in range(C):
                osb = outsb_pool.tile([P, D], F32, tag="outsb",
                                      name=f"o{c}" + sfx)
                for q in range(NQ):
                    osl = osb[:, q * QS:(q + 1) * QS]
                    x1l = x11[:, q * QS:(q + 1) * QS]
                    if post_bc is None:
                        # out = agg * rstd_agg + x_cur
                        nc.vector.scalar_tensor_tensor(
                            out=osl, in0=aggsb[(c, q)],
                            scalar=rstda[:, c:c + 1], in1=x1l,
                            op0=OP.mult, op1=OP.add)
                    else:
                        nc.vector.tensor_scalar(
                            out=osl, in0=aggsb[(c, q)],
                            scalar1=rstda[:, c:c + 1], scalar2=None,
                            op0=OP.mult)
                        nc.vector.tensor_mul(
                            osl, osl, post_bc[:, c, q * QS:(q + 1) * QS])
                        nc.vector.tensor_add(osl, osl, x1l)
                nc.sync.dma_start(out=out_s[c, p0:p0 + P, :], in_=osb)

    nc.compile()
    return nc


def _in_maps(inputs: dict) -> tuple[bass.Bass, list[dict]]:
    xs = np.ascontiguousarray(inputs["xs"], dtype=np.float32)
    w1 = np.ascontiguousarray(inputs["w1"], dtype=np.float32)
    w2 = np.ascontiguousarray(inputs["w2"], dtype=np.float32)
    stat = np.ascontiguousarray(inputs["static_weight"], dtype=np.float32)
    pre = np.ascontiguousarray(inputs["pre_scales"], dtype=np.float32)
    post = np.ascontiguousarray(inputs["post_scales"], dtype=np.float32)

    pre_ones = bool(np.all(pre == 1.0))
    post_ones = bool(np.all(post == 1.0))
    nc = build_kernel(pre_ones, post_ones)

    xs_flat = xs.reshape(L, B * T, D)
    import ml_dtypes
    ident = np.eye(P, dtype=np.float32)
    identb = np.eye(P, dtype=ml_dtypes.bfloat16)
    maps = []
    for k in range(NCORES):
        m = {
            "xs_s": np.ascontiguousarray(xs_flat[:, k * NPOS:(k + 1) * NPOS, :]),
            "w1": w1,
            "w2": w2,
            "stat": stat.reshape(CL),
            "ident": ident,
            "identb": identb,
        }
        if not pre_ones:
            m["pre"] = pre
        if not post_ones:
            m["post"] = post
        maps.append(m)
    return nc, maps


def kernel(**inputs) -> np.ndarray:
    global _LAST_RESULTS
    nc, maps = _in_maps(inputs)
    trace = os.environ.get("KERNEL_TRACE", "0") == "1"
    res = run_bass_kernel_spmd(nc, maps, core_ids=list(range(NCORES)),
                               trace=trace)
    _LAST_RESULTS = res
    out = np.concatenate([r["out_s"] for r in res.results], axis=1)
    return np.ascontiguousarray(out.reshape(C, B, T, D))


# revision 16
# speedup vs baseline: 1.2573x; 1.0643x over previous
"""Trainium2 Bass kernel for DepthAggregateModule.

Reference computation (per position p = (b,t), D=2048, L=12, C=4, H=48):
  x_cur   = xs[L-1]
  dw      = gelu(rms(x_cur) @ w1) @ w2            # [pos, C*L]
  dw      = dw + static_weight  (flat col c*L+l)
  agg[c]  = sum_l dw[:, c*L+l] * (rms(xs_l) * pre_scales[l])   # [pos, D]
  out[c]  = rms(agg[c]) * post_scales[c] + x_cur

Strategy: shard the 4096 (b,t) positions across 8 cores (512 each).
Per core, 4 position-tiles of 128 (partition dim = position).  The
per-position l-contraction runs on the TensorEngine as diagonal
matmuls: lhsT = diag(dw[:, c*L+l] * rstd_l) [128x128] bf16, rhs =
xs_l chunk [128x512] bf16, accumulated over l in fp32 PSUM.  The rms
scale factors fold into the diagonals, so bulk xs data is touched only
by: casting DMA-in (f32->bf16 via SWDGE), one ACT square+accumulate
pass (row sum-of-squares), and the PE einsum stream.  Layer L-1 is
also loaded in f32 for the residual add and the dw matmul chain.
"""

import os
from contextlib import ExitStack

import numpy as np

import concourse.bacc as bacc
import concourse.bass as bass
import concourse.tile as tile
from concourse import mybir
from concourse.bass_utils import run_bass_kernel_spmd

F32 = mybir.dt.float32
BF16 = mybir.dt.bfloat16
AF = mybir.ActivationFunctionType
OP = mybir.AluOpType
AX = mybir.AxisListType

L, B, T, D = 12, 2, 2048, 2048
C, H = 4, 48
CL = C * L
EPS = 1e-6
NCORES = 8
P = 128                      # partition dim = positions per tile
NPOS = (B * T) // NCORES     # 512 positions per core
NTILES = NPOS // P           # 4
QS = 512                     # free-dim chunk (one fp32 PSUM bank)
NQ = D // QS                 # 4

_LAST_RESULTS = None         # BassKernelResults of the last run (for test.py)


def build_kernel(pre_ones: bool, post_ones: bool) -> bass.Bass:
    nc = bacc.Bacc("TRN2", target_bir_lowering=False, debug=False)

    xs_s = nc.dram_tensor("xs_s", [L, NPOS, D], F32, kind="ExternalInput")
    w1_t = nc.dram_tensor("w1", [D, H], F32, kind="ExternalInput")
    w2_t = nc.dram_tensor("w2", [H, CL], F32, kind="ExternalInput")
    stat_t = nc.dram_tensor("stat", [CL], F32, kind="ExternalInput")
    ident_t = nc.dram_tensor("ident", [P, P], F32, kind="ExternalInput")
    identb_t = nc.dram_tensor("identb", [P, P], BF16, kind="ExternalInput")
    pre_t = None if pre_ones else nc.dram_tensor("pre", [L, D], F32, kind="ExternalInput")
    post_t = None if post_ones else nc.dram_tensor("post", [C, D], F32, kind="ExternalInput")
    out_s = nc.dram_tensor("out_s", [C, NPOS, D], F32, kind="ExternalOutput")

    with ExitStack() as ctx:
        tc = ctx.enter_context(tile.TileContext(nc))

        consts = ctx.enter_context(tc.tile_pool(name="consts", bufs=1))
        xs_pool = ctx.enter_context(tc.tile_pool(name="xs", bufs=2))
        sm_pool = ctx.enter_context(tc.tile_pool(name="small", bufs=2))
        diag_pool = ctx.enter_context(tc.tile_pool(name="diag", bufs=2))
        outsb_pool = ctx.enter_context(tc.tile_pool(name="outsb", bufs=3))
        scr_pool = ctx.enter_context(tc.tile_pool(name="scr", bufs=1))
        agg_ps = ctx.enter_context(tc.tile_pool(name="aggps", bufs=1, space="PSUM"))
        pt_ps = ctx.enter_context(tc.tile_pool(name="ptps", bufs=1, space="PSUM"))
        dw_ps = ctx.enter_context(tc.tile_pool(name="dwps", bufs=1, space="PSUM"))

        # ---- constants -------------------------------------------------
        ident_sb = consts.tile([P, P], F32, name="ident_sb")
        nc.sync.dma_start(out=ident_sb, in_=ident_t[:, :])
        identb_sb = consts.tile([P, P], BF16, name="identb_sb")
        nc.sync.dma_start(out=identb_sb, in_=identb_t[:, :])

        w1s = consts.tile([P, D // P, H], F32, name="w1s")
        nc.sync.dma_start(
            out=w1s, in_=w1_t[:, :].rearrange("(k p) h -> p k h", p=P)
        )

        w2s = consts.tile([H, CL], F32, name="w2s")
        nc.sync.dma_start(out=w2s, in_=w2_t[:, :])

        stat_bc = consts.tile([P, CL], F32, name="stat_bc")
        stat_ap = stat_t[:]
        nc.sync.dma_start(
            out=stat_bc,
            in_=bass.AP(tensor=stat_ap.tensor, offset=stat_ap.offset,
                        ap=[[0, P]] + stat_ap.ap),
        )

        eps_sb = consts.tile([P, 1], F32, name="eps_sb")
        nc.vector.memset(eps_sb, EPS)

        pre_bc = None
        if pre_t is not None:
            # [128, L, D] replicated across partitions: 12 MB — fallback only.
            pre_bc = consts.tile([P, L, D], F32, name="pre_bc")
            pre_ap = pre_t[:, :]
            nc.sync.dma_start(
                out=pre_bc,
                in_=bass.AP(tensor=pre_ap.tensor, offset=pre_ap.offset,
                            ap=[[0, P]] + pre_ap.ap),
            )
        post_bc = None
        if post_t is not None:
            post_bc = consts.tile([P, C, D], F32, name="post_bc")
            post_ap = post_t[:, :]
            nc.sync.dma_start(
                out=post_bc,
                in_=bass.AP(tensor=post_ap.tensor, offset=post_ap.offset,
                            ap=[[0, P]] + post_ap.ap),
            )

        HS = 1024                                       # sq-accum chunk
        scr = scr_pool.tile([P, HS], F32, name="scr")   # ACT dump target

        # ---- per position-tile ----------------------------------------
        for it in range(NTILES):
            p0 = it * P
            sfx = f"_{it}"

            xb = [None] * L          # bf16 [P, D] per layer (einsum rhs)
            rstd = sm_pool.tile([P, L], F32, tag="rstd", name="rstd" + sfx)
            rsr = sm_pool.tile([P, L], F32, tag="rsr", name="rsr" + sfx)

            def rstd_from_sq(l, sq):
                # rstd_l = 1/sqrt(mean(x^2) + eps)
                nc.vector.tensor_reduce(out=rsr[:, l:l + 1], in_=sq,
                                        axis=AX.X, op=OP.add)
                nc.scalar.activation(out=rsr[:, l:l + 1], in_=rsr[:, l:l + 1],
                                     func=AF.Sqrt, bias=eps_sb, scale=1.0 / D)
                nc.vector.reciprocal(out=rstd[:, l:l + 1], in_=rsr[:, l:l + 1])

            # --- layer L-1 in f32 (residual + dw chain), plus bf16 copy.
            sq11 = sm_pool.tile([P, 2], F32, tag="sq11", name="sq11" + sfx)
            xb11 = xs_pool.tile([P, D], BF16, tag="xb11", name="xb11" + sfx)
            x11 = xs_pool.tile([P, D], F32, tag="x11", name="x11" + sfx)
            nc.sync.dma_start(out=x11, in_=xs_s[L - 1, p0:p0 + P, :])
            for h in range(2):
                nc.scalar.activation(out=scr, in_=x11[:, h * HS:(h + 1) * HS],
                                     func=AF.Square,
                                     accum_out=sq11[:, h:h + 1])
            nc.vector.tensor_copy(out=xb11, in_=x11)
            xb[L - 1] = xb11
            rstd_from_sq(L - 1, sq11)

            # Transpose x_cur into [d, pos] chunks for the w1 matmul.
            x11T = sm_pool.tile([P, D // P, P], F32, tag="x11T",
                                name="x11T" + sfx)
            for q in range(NQ):
                pt = pt_ps.tile([P, QS], F32, tag="pt", name=f"pt{q}" + sfx)
                for j in range(4):
                    k = q * 4 + j
                    nc.tensor.matmul(
                        pt[:, j * P:(j + 1) * P],
                        x11[:, k * P:(k + 1) * P],
                        ident_sb, is_transpose=True, skip_group_check=True)
                    nc.vector.tensor_copy(out=x11T[:, k, :],
                                          in_=pt[:, j * P:(j + 1) * P])

            # dw chain: dw = gelu(rms(x_cur) @ w1) @ w2 + static
            pdw1 = dw_ps.tile([P, H], F32, tag="pdw", name="pdw1" + sfx)
            for k in range(D // P):
                nc.tensor.matmul(pdw1, x11T[:, k, :], w1s[:, k, :],
                                 start=(k == 0), stop=(k == D // P - 1))
            h1 = sm_pool.tile([P, H], F32, tag="h1", name="h1" + sfx)
            nc.scalar.activation(out=h1, in_=pdw1, func=AF.Gelu,
                                 scale=rstd[:, L - 1:L])
            pth = pt_ps.tile([P, QS], F32, tag="pt", name="pth" + sfx)
            nc.tensor.matmul(pth[:H, :P], h1, ident_sb, is_transpose=True,
                             skip_group_check=True)
            h1T = sm_pool.tile([H, P], F32, tag="h1T", name="h1T" + sfx)
            nc.vector.tensor_copy(out=h1T, in_=pth[:H, :P])
            pdw2 = dw_ps.tile([P, CL], F32, tag="pdw", name="pdw2" + sfx)
            nc.tensor.matmul(pdw2, h1T, w2s, start=True, stop=True)
            dwf = sm_pool.tile([P, CL], F32, tag="dwf", name="dwf" + sfx)
            nc.vector.scalar_tensor_tensor(out=dwf, in0=pdw2, scalar=1.0,
                                           in1=stat_bc, op0=OP.mult,
                                           op1=OP.add)

            # --- layers 0..L-2: casting DMA straight to bf16.
            for l in range(L - 1):
                xbl = xs_pool.tile([P, D], BF16, tag=f"xb{l}",
                                   name=f"xb{l}" + sfx)
                nc.gpsimd.dma_start(out=xbl, in_=xs_s[l, p0:p0 + P, :])
                sq = sm_pool.tile([P, 2], F32, tag=f"sq{l}",
                                  name=f"sq{l}" + sfx, bufs=1)
                for h in range(2):
                    nc.scalar.activation(out=scr,
                                         in_=xbl[:, h * HS:(h + 1) * HS],
                                         func=AF.Square,
                                         accum_out=sq[:, h:h + 1])
                rstd_from_sq(l, sq)
                xb[l] = xbl

            # Fallback: apply pre_scales in place (after the sq-accum).
            if pre_bc is not None:
                for l in range(L):
                    nc.vector.tensor_mul(xb[l], xb[l], pre_bc[:, l, :])

            # Diagonals: diag_cl = ident * dw[:, c*L+l] * rstd_l  (bf16)
            diag = [[None] * L for _ in range(C)]
            for c in range(C):
                for l in range(L):
                    dg = diag_pool.tile([P, P], BF16, tag=f"dg{c}_{l}",
                                        name=f"dg{c}_{l}" + sfx)
                    nc.vector.tensor_scalar(
                        out=dg, in0=identb_sb,
                        scalar1=dwf[:, c * L + l:c * L + l + 1],
                        scalar2=rstd[:, l:l + 1],
                        op0=OP.mult, op1=OP.mult)
                    diag[c][l] = dg

            # Einsum: agg[c] = sum_l diag_cl @ xs_l (fp32 PSUM), then the
            # post-RMS + residual reads PSUM directly.  c is processed
            # serially so 4 q-banks per c (x2 buffered on q0/q1) fit PSUM.
            aggsq = [sm_pool.tile([P, NQ], F32, tag=f"asq{c}",
                                  name=f"asq{c}" + sfx) for c in range(C)]
            rstda = sm_pool.tile([P, C], F32, tag="rstda", name="rstda" + sfx)
            rsra = sm_pool.tile([P, C], F32, tag="rsra", name="rsra" + sfx)
            for c in range(C):
                osb = outsb_pool.tile([P, D], F32, tag="outsb",
                                      name=f"o{c}" + sfx)
                paggs = []
                for q in range(NQ):
                    pagg = agg_ps.tile([P, QS], F32, tag=f"agg{q}",
                                       name=f"pagg{c}_{q}" + sfx,
                                       bufs=2 if q < 2 else 1)
                    for l in range(L):
                        nc.tensor.matmul(
                            pagg, diag[c][l],
                            xb[l][:, q * QS:(q + 1) * QS],
                            start=(l == 0), stop=(l == L - 1))
                    nc.scalar.activation(out=scr[:, :QS], in_=pagg,
                                         func=AF.Square,
                                         accum_out=aggsq[c][:, q:q + 1])
                    paggs.append(pagg)
                nc.vector.tensor_reduce(out=rsra[:, c:c + 1], in_=aggsq[c],
                                        axis=AX.X, op=OP.add)
                nc.scalar.activation(out=rsra[:, c:c + 1], in_=rsra[:, c:c + 1],
                                     func=AF.Sqrt, bias=eps_sb, scale=1.0 / D)
                nc.vector.reciprocal(out=rstda[:, c:c + 1], in_=rsra[:, c:c + 1])
                for q in range(NQ):
                    osl = osb[:, q * QS:(q + 1) * QS]
                    x1l = x11[:, q * QS:(q + 1) * QS]
                    if post_bc is None:
                        # out = agg * rstd_agg + x_cur
                        nc.vector.scalar_tensor_tensor(
                            out=osl, in0=paggs[q],
                            scalar=rstda[:, c:c + 1], in1=x1l,
                            op0=OP.mult, op1=OP.add)
                    else:
                        nc.vector.tensor_scalar(
                            out=osl, in0=paggs[q],
                            scalar1=rstda[:, c:c + 1], scalar2=None,
                            op0=OP.mult)
                        nc.vector.tensor_mul(
                            osl, osl, post_bc[:, c, q * QS:(q + 1) * QS])
                        nc.vector.tensor_add(osl, osl, x1l)
                nc.sync.dma_start(out=out_s[c, p0:p0 + P, :], in_=osb)

    nc.compile()
    return nc


def _in_maps(inputs: dict) -> tuple[bass.Bass, list[dict]]:
    xs = np.ascontiguousarray(inputs["xs"], dtype=np.float32)
    w1 = np.ascontiguousarray(inputs["w1"], dtype=np.float32)
    w2 = np.ascontiguousarray(inputs["w2"], dtype=np.float32)
    stat = np.ascontiguousarray(inputs["static_weight"], dtype=np.float32)
    pre = np.ascontiguousarray(inputs["pre_scales"], dtype=np.float32)
    post = np.ascontiguousarray(inputs["post_scales"], dtype=np.float32)

    pre_ones = bool(np.all(pre == 1.0))
    post_ones = bool(np.all(post == 1.0))
    nc = build_kernel(pre_ones, post_ones)

    xs_flat = xs.reshape(L, B * T, D)
    import ml_dtypes
    ident = np.eye(P, dtype=np.float32)
    identb = np.eye(P, dtype=ml_dtypes.bfloat16)
    maps = []
    for k in range(NCORES):
        m = {
            "xs_s": np.ascontiguousarray(xs_flat[:, k * NPOS:(k + 1) * NPOS, :]),
            "w1": w1,
            "w2": w2,
            "stat": stat.reshape(CL),
            "ident": ident,
            "identb": identb,
        }
        if not pre_ones:
            m["pre"] = pre
        if not post_ones:
            m["post"] = post
        maps.append(m)
    return nc, maps


def kernel(**inputs) -> np.ndarray:
    global _LAST_RESULTS
    nc, maps = _in_maps(inputs)
    trace = os.environ.get("KERNEL_TRACE", "0") == "1"
    res = run_bass_kernel_spmd(nc, maps, core_ids=list(range(NCORES)),
                               trace=trace)
    _LAST_RESULTS = res
    out = np.concatenate([r["out_s"] for r in res.results], axis=1)
    return np.ascontiguousarray(out.reshape(C, B, T, D))
